# revision 13
# baseline (speedup 1.0000x reference)
"""Masked attention (out, p_attn) kernel for Trainium2, 8-core SPMD.

Problem: B=2 H=8 N=4096 D=64 fp32 attention with outer-product int mask,
returning BOTH the attention output [B,H,N,D] and the full probability
matrix p_attn [B,H,N,N] (the 1.07 GB p_attn write dominates -> memory
bound, roofline ~400us across 8 cores).

Sharding: the 16 (b,h) heads are split 2-per-core (head-parallel, no
communication).

Per-core pipeline per head (matmul operands are float32r: fp32 storage
rounded so the PE runs 1 cycle/row; ~1.5e-4 matmul rel err vs bf16's
2.3e-3):
  - Host augments Q with a ones column and K with a mask-bias column
    ((mask_j-1)*8e9), so the single QK matmul (K=65 contraction) yields
    raw scores with -1e9 (pre-scale) already added to masked key columns.
    V gets a ones column, so the PV matmul's row 64 accumulates the
    softmax denominator Z for free.
  - Prologue PE-transposes Q,K into d-major [65, N] f32r operands.
  - Per 128-row i-chunk: S tiles in PSUM; ACT exp with per-partition
    scale = 0.125 * mask_i (masked query rows -> exp(0)=1 everywhere ->
    uniform row, matching jax softmax of an all--1e9 row). exp output is
    unnormalized P (f32r).
  - PE transposes P 128x128 -> PSUM; ACT/DVE copy tiles back to SBUF
    (PT). PV matmul outT[65, 256] = sum_j V'[j,:].T @ PT[j, i-range].
  - Fixup: PE-transpose outT -> [128, 65]; col 64 = Z; DVE reciprocal
    gives 1/Z per query row, scales out rows, and normalizes P for the
    p_attn DMA.
"""

import numpy as np

import concourse.bass as bass
import concourse.mybir as mybir
import concourse.tile as tile
from concourse import bacc
from concourse.bass import ds, ts
from concourse.bass_utils import run_bass_kernel_spmd
from concourse.masks import make_identity

dt = mybir.dt
AF = mybir.ActivationFunctionType

B, H, N, D = 2, 8, 4096, 64
N_CORES = 8
HPC = (B * H) // N_CORES  # heads per core
DA = D + 1  # augmented feature dim


def build_nc(n_ctx=N, heads=HPC, act_copy_mod=3):
    """Build + compile the per-core kernel for `heads` heads of [n_ctx, D]."""
    nch = n_ctx // 128
    n_jq = max(1, n_ctx // 1024)
    jt = n_ctx // n_jq
    assert jt <= 1024 and jt % 512 == 0

    nc = bacc.Bacc("TRN2", target_bir_lowering=False, debug=False)
    q_d = nc.dram_tensor("q", [heads, 128, nch, DA], dt.float32, kind="ExternalInput").ap()
    k_d = nc.dram_tensor("k", [heads, 128, nch, DA], dt.float32, kind="ExternalInput").ap()
    v_d = nc.dram_tensor("v", [heads, 128, nch, DA], dt.float32, kind="ExternalInput").ap()
    m_d = nc.dram_tensor("m", [heads, nch, 128], dt.int32, kind="ExternalInput").ap()
    out_d = nc.dram_tensor("out", [heads, nch, 128, D], dt.float32, kind="ExternalOutput").ap()
    pat_d = nc.dram_tensor("pat", [heads, nch, 128, n_ctx], dt.float32, kind="ExternalOutput").ap()

    with tile.TileContext(nc) as tc:
        with (
            tc.tile_pool(name="const", bufs=1) as constp,
            tc.tile_pool(name="loads", bufs=1) as loads,
            tc.tile_pool(name="opnds", bufs=1) as opnds,
            tc.tile_pool(name="vr", bufs=1) as vrp,
            tc.tile_pool(name="prob", bufs=3) as prob,
            tc.tile_pool(name="pnorm", bufs=2) as pnorm,
            tc.tile_pool(name="ptsb", bufs=1) as ptsb,
            tc.tile_pool(name="stats", bufs=2) as stats,
            tc.tile_pool(name="outsb", bufs=2) as outsb,
            tc.tile_pool(name="psS", bufs=2, space="PSUM") as psS,
            tc.tile_pool(name="psPT", bufs=3, space="PSUM") as psPT,
            tc.tile_pool(name="psO", bufs=1, space="PSUM") as psO,
        ):
            ident_f = constp.tile([128, 128], dt.float32)
            make_identity(nc, ident_f[:])
            ident_r = constp.tile([128, 128], dt.float32r)
            nc.vector.tensor_copy(ident_r[:], ident_f[:])

            for h in range(heads):
                # ---- loads ----
                qf = loads.tile([128, nch, DA], dt.float32, tag="qf")
                kf = loads.tile([128, nch, DA], dt.float32, tag="kf")
                vf = loads.tile([128, nch, DA], dt.float32, tag="vf")
                nc.sync.dma_start(qf[:], q_d[h])
                nc.sync.dma_start(kf[:], k_d[h])
                nc.sync.dma_start(vf[:], v_d[h])
                m32 = loads.tile([nch, 128], dt.int32, tag="m32")
                nc.sync.dma_start(m32[:], m_d[h])

                # ---- operand prep ----
                v_r = vrp.tile([128, nch, DA], dt.float32r, tag="vr")
                nc.vector.tensor_copy(v_r[:], vf[:])

                qt = opnds.tile([DA, n_ctx], dt.float32r, tag="qt")
                kt = opnds.tile([DA, n_ctx], dt.float32r, tag="kt")
                for src, dst in ((qf, qt), (kf, kt)):
                    for g in range(nch // 4):
                        ptile = psPT.tile([DA, 512], dt.float32, tag="PT")
                        for t in range(4):
                            c = g * 4 + t
                            nc.tensor.transpose(
                                ptile[:, ts(t, 128)], src[:, c, :], ident_f[:]
                            )
                        nc.vector.tensor_copy(dst[:, ts(g, 512)], ptile[:])

                # mask_sc[p, ic] = 0.125 * mask_i  (per-partition exp scale)
                m32f = stats.tile([nch, 128], dt.float32, tag="m32f")
                nc.vector.tensor_copy(m32f[:], m32[:])
                mfix = psO.tile([128, DA], dt.float32, tag="obank")
                nc.tensor.transpose(
                    mfix[:, 0:nch], m32f[:], ident_f[0:nch, 0:nch]
                )
                mask_sc = stats.tile([128, nch], dt.float32, tag="mask_sc")
                nc.vector.tensor_scalar_mul(mask_sc[:], mfix[:, 0:nch], 0.125)

                pt_sb = ptsb.tile([128, nch, 256], dt.float32r, tag="pt")
                p_tiles = {}

                def emit_qk_exp(ic):
                    p_r = prob.tile([128, n_ctx], dt.float32r, tag="p")
                    p_tiles[ic] = p_r
                    for jq in range(n_jq):
                        s_ps = psS.tile([128, jt], dt.float32, tag="S")
                        for js in range(jt // 512):
                            nc.tensor.matmul(
                                s_ps[:, ts(js, 512)],
                                qt[:, ts(ic, 128)],
                                kt[:, ds(jq * jt + js * 512, 512)],
                                start=True, stop=True,
                            )
                        nc.scalar.activation(
                            p_r[:, ts(jq, jt)], s_ps[:], AF.Exp,
                            scale=mask_sc[:, ic:ic + 1],
                        )

                def emit_transpose_group(ic, g):
                    p_r = p_tiles[ic]
                    ptp = psPT.tile([128, 512], dt.float32r, tag="PT")
                    for t in range(4):
                        nc.tensor.transpose(
                            ptp[:, ts(t, 128)],
                            p_r[:, ts(g * 4 + t, 128)],
                            ident_r[:],
                        )
                    cb_out = pt_sb[:, g * 4:g * 4 + 4, ds((ic % 2) * 128, 128)]
                    if g % act_copy_mod == 0:
                        nc.scalar.copy(cb_out, ptp[:])
                    else:
                        nc.vector.tensor_copy(cb_out, ptp[:])

                def emit_transposes(ic):
                    for g in range(nch // 4):
                        emit_transpose_group(ic, g)

                pv_state = {"jc": 0, "ot": None}

                def emit_pv_upto(jc_end):
                    # emit PV accumulation matmuls for jc in [state, jc_end)
                    if pv_state["jc"] == 0 and jc_end > 0:
                        ot_new = psO.tile([DA, 256], dt.float32, tag="obank")
                        pv_state["ot"] = ot_new
                    ot_ps = pv_state["ot"]
                    for jc in range(pv_state["jc"], jc_end):
                        nc.tensor.matmul(
                            ot_ps[:],
                            v_r[:, jc, :],
                            pt_sb[:, jc, :],
                            start=(jc == 0), stop=(jc == nch - 1),
                        )
                    pv_state["jc"] = jc_end

                def emit_pv(ic1):
                    # epilogue: consumes accumulated outT for (ic1-1, ic1)
                    emit_pv_upto(nch)
                    pv_state["jc"] = 0
                    ot_ps = pv_state["ot"]
                    ot_sb = outsb.tile([DA, 256], dt.float32, tag="ot_sb")
                    nc.vector.tensor_copy(ot_sb[:], ot_ps[:])
                    for t in range(2):
                        i_abs = (ic1 - 1) + t
                        fix = psO.tile([128, DA], dt.float32, tag="obank")
                        nc.tensor.transpose(
                            fix[:], ot_sb[:, ts(t, 128)],
                            ident_f[0:DA, 0:DA],
                        )
                        recz = stats.tile([128, 1], dt.float32, tag="recz")
                        nc.vector.reciprocal(recz[:], fix[:, D:DA])
                        o_sb = outsb.tile([128, D], dt.float32, tag="o_sb")
                        nc.vector.tensor_scalar_mul(o_sb[:], fix[:, 0:D], recz[:])
                        nc.sync.dma_start(out_d[h, i_abs], o_sb[:])
                        pn = pnorm.tile([128, n_ctx], dt.float32, tag="pn")
                        nc.vector.tensor_scalar_mul(
                            pn[:], p_tiles[i_abs][:], recz[:]
                        )
                        nc.sync.dma_start(pat_d[h, i_abs], pn[:])
                    del p_tiles[ic1 - 1], p_tiles[ic1]

                # Interleave PV chunks (HAM-counted matmuls) between
                # transpose groups (transpose-mode is not HAM-counted) and
                # spread each pair's PV over BOTH following transpose
                # stretches: first half during the odd i-chunk, second half
                # during the next even i-chunk.
                ng = nch // 4
                for ic in range(nch):
                    emit_qk_exp(ic)
                    for g in range(ng):
                        if ic % 2 == 0 and ic > 0:
                            # finish previous pair's PV, paced through this
                            # even stretch but always ahead of the copyback
                            # that overwrites the pt_sb half it reads
                            emit_pv_upto(min(nch, nch // 2 + 4 * (g + 1)))
                        emit_transpose_group(ic, g)
                        if ic % 2 == 1 and g % 2 == 1:
                            # first half of this pair's PV over the odd stretch
                            emit_pv_upto((g + 1) * nch // (2 * ng))
                    if ic % 2 == 0 and ic > 0:
                        emit_pv(ic - 1)
                emit_pv(nch - 1)

    nc.compile()
    return nc


_cache = {}


def get_nc(n_ctx=N, heads=HPC):
    key = (n_ctx, heads)
    if key not in _cache:
        _cache[key] = build_nc(n_ctx, heads)
    return _cache[key]


def _augment(x, aug_col):
    """[HPC, n, D] + [HPC, n] -> [HPC, 128, nch, D+1] chunk layout."""
    hpc, n, d = x.shape
    nch = n // 128
    xa = np.concatenate([x, aug_col[:, :, None].astype(np.float32)], axis=-1)
    return np.ascontiguousarray(
        xa.reshape(hpc, nch, 128, d + 1).transpose(0, 2, 1, 3)
    )


def make_in_maps(query, key, value, mask):
    q = np.asarray(query, dtype=np.float32).reshape(B * H, N, D)
    k = np.asarray(key, dtype=np.float32).reshape(B * H, N, D)
    v = np.asarray(value, dtype=np.float32).reshape(B * H, N, D)
    m = np.asarray(mask, dtype=np.int32)
    nch = N // 128

    ones = np.ones((HPC, N), np.float32)
    in_maps = []
    for c in range(N_CORES):
        gs = [HPC * c + i for i in range(HPC)]
        sl = slice(gs[0], gs[-1] + 1)
        mh = np.stack([m[g // H] for g in gs])  # [HPC, N] int32
        mbias = (mh.astype(np.float32) - 1.0) * 8.0e9
        in_maps.append({
            "q": _augment(q[sl], ones),
            "k": _augment(k[sl], mbias),
            "v": _augment(v[sl], ones),
            "m": np.ascontiguousarray(mh.reshape(HPC, nch, 128)),
        })
    return in_maps


def gather_results(res):
    out = np.concatenate([r["out"] for r in res]).reshape(B, H, N, D)
    pat = np.concatenate([r["pat"] for r in res]).reshape(B, H, N, N)
    return out, pat


def kernel(query, key, value, mask):
    nc = get_nc()
    in_maps = make_in_maps(query, key, value, mask)
    res = run_bass_kernel_spmd(nc, in_maps, core_ids=list(range(N_CORES))).results
    return gather_results(res)


# revision 14
# speedup vs baseline: 1.0945x; 1.0945x over previous
"""Masked attention (out, p_attn) kernel for Trainium2, 8-core SPMD.

Problem: B=2 H=8 N=4096 D=64 fp32 attention with outer-product int mask,
returning BOTH the attention output [B,H,N,D] and the full probability
matrix p_attn [B,H,N,N] (the 1.07 GB p_attn write dominates -> memory
bound, roofline ~400us across 8 cores).

Sharding: the 16 (b,h) heads are split 2-per-core (head-parallel, no
communication).

Per-core pipeline per head (matmul operands are float32r: fp32 storage
rounded so the PE runs 1 cycle/row; ~1.5e-4 matmul rel err vs bf16's
2.3e-3):
  - Host augments Q with a ones column and K with a mask-bias column
    ((mask_j-1)*8e9), so the single QK matmul (K=65 contraction) yields
    raw scores with -1e9 (pre-scale) already added to masked key columns.
    V gets a ones column, so the PV matmul's row 64 accumulates the
    softmax denominator Z for free.
  - Prologue PE-transposes Q,K into d-major [65, N] f32r operands.
  - Per 128-row i-chunk: S tiles in PSUM; ACT exp with per-partition
    scale = 0.125 * mask_i (masked query rows -> exp(0)=1 everywhere ->
    uniform row, matching jax softmax of an all--1e9 row). exp output is
    unnormalized P (f32r).
  - PE transposes P 128x128 -> PSUM; ACT/DVE copy tiles back to SBUF
    (PT). PV matmul outT[65, 256] = sum_j V'[j,:].T @ PT[j, i-range].
  - Fixup: PE-transpose outT -> [128, 65]; col 64 = Z; DVE reciprocal
    gives 1/Z per query row, scales out rows, and normalizes P for the
    p_attn DMA.
"""

import numpy as np

import concourse.bass as bass
import concourse.mybir as mybir
import concourse.tile as tile
from concourse import bacc
from concourse.bass import ds, ts
from concourse.bass_utils import run_bass_kernel_spmd
from concourse.masks import make_identity

dt = mybir.dt
AF = mybir.ActivationFunctionType

B, H, N, D = 2, 8, 4096, 64
N_CORES = 8
HPC = (B * H) // N_CORES  # heads per core
DA = D + 1  # augmented feature dim


def build_nc(n_ctx=N, heads=HPC, act_copy_mod=3):
    """Build + compile the per-core kernel for `heads` heads of [n_ctx, D]."""
    nch = n_ctx // 128
    n_jq = max(1, n_ctx // 1024)
    jt = n_ctx // n_jq
    assert jt <= 1024 and jt % 512 == 0

    nc = bacc.Bacc("TRN2", target_bir_lowering=False, debug=False)
    q_d = nc.dram_tensor("q", [heads, 128, nch, DA], dt.float32, kind="ExternalInput").ap()
    k_d = nc.dram_tensor("k", [heads, 128, nch, DA], dt.float32, kind="ExternalInput").ap()
    v_d = nc.dram_tensor("v", [heads, 128, nch, DA], dt.float32, kind="ExternalInput").ap()
    m_d = nc.dram_tensor("m", [heads, nch, 128], dt.int32, kind="ExternalInput").ap()
    out_d = nc.dram_tensor("out", [heads, nch, 128, D], dt.float32, kind="ExternalOutput").ap()
    pat_d = nc.dram_tensor("pat", [heads, nch, 128, n_ctx], dt.float32, kind="ExternalOutput").ap()

    with tile.TileContext(nc) as tc:
        with (
            tc.tile_pool(name="const", bufs=1) as constp,
            tc.tile_pool(name="loads", bufs=1) as loads,
            tc.tile_pool(name="opnds", bufs=1) as opnds,
            tc.tile_pool(name="vr", bufs=1) as vrp,
            tc.tile_pool(name="prob", bufs=3) as prob,
            tc.tile_pool(name="pnorm", bufs=2) as pnorm,
            tc.tile_pool(name="ptsb", bufs=1) as ptsb,
            tc.tile_pool(name="stats", bufs=2) as stats,
            tc.tile_pool(name="outsb", bufs=2) as outsb,
            tc.tile_pool(name="psS", bufs=2, space="PSUM") as psS,
            tc.tile_pool(name="psPT", bufs=3, space="PSUM") as psPT,
            tc.tile_pool(name="psO", bufs=1, space="PSUM") as psO,
        ):
            ident_f = constp.tile([128, 128], dt.float32)
            make_identity(nc, ident_f[:])
            ident_r = constp.tile([128, 128], dt.float32r)
            nc.vector.tensor_copy(ident_r[:], ident_f[:])

            for h in range(heads):
                # ---- loads ----
                qf = loads.tile([128, nch, DA], dt.float32, tag="qf")
                kf = loads.tile([128, nch, DA], dt.float32, tag="kf")
                vf = loads.tile([128, nch, DA], dt.float32, tag="vf")
                nc.sync.dma_start(qf[:], q_d[h])
                nc.sync.dma_start(kf[:], k_d[h])
                nc.sync.dma_start(vf[:], v_d[h])
                m32 = loads.tile([nch, 128], dt.int32, tag="m32")
                nc.sync.dma_start(m32[:], m_d[h])

                # ---- operand prep ----
                v_r = vrp.tile([128, nch, DA], dt.float32r, tag="vr")
                nc.vector.tensor_copy(v_r[:], vf[:])

                qt = opnds.tile([DA, n_ctx], dt.float32r, tag="qt")
                kt = opnds.tile([DA, n_ctx], dt.float32r, tag="kt")
                for src, dst in ((qf, qt), (kf, kt)):
                    for g in range(nch // 4):
                        ptile = psPT.tile([DA, 512], dt.float32, tag="PT")
                        for t in range(4):
                            c = g * 4 + t
                            nc.tensor.transpose(
                                ptile[:, ts(t, 128)], src[:, c, :], ident_f[:]
                            )
                        nc.vector.tensor_copy(dst[:, ts(g, 512)], ptile[:])

                # mask_sc[p, ic] = 0.125 * mask_i  (per-partition exp scale)
                m32f = stats.tile([nch, 128], dt.float32, tag="m32f")
                nc.vector.tensor_copy(m32f[:], m32[:])
                mfix = psO.tile([128, DA], dt.float32, tag="obank")
                nc.tensor.transpose(
                    mfix[:, 0:nch], m32f[:], ident_f[0:nch, 0:nch]
                )
                mask_sc = stats.tile([128, nch], dt.float32, tag="mask_sc")
                nc.vector.tensor_scalar_mul(mask_sc[:], mfix[:, 0:nch], 0.125)

                pt_sb = ptsb.tile([128, nch, 256], dt.float32r, tag="pt")
                p_tiles = {}

                def emit_qk_exp(ic):
                    p_r = prob.tile([128, n_ctx], dt.float32r, tag="p")
                    p_tiles[ic] = p_r
                    for jq in range(n_jq):
                        s_ps = psS.tile([128, jt], dt.float32, tag="S")
                        for js in range(jt // 512):
                            nc.tensor.matmul(
                                s_ps[:, ts(js, 512)],
                                qt[:, ts(ic, 128)],
                                kt[:, ds(jq * jt + js * 512, 512)],
                                start=True, stop=True,
                            )
                        nc.scalar.activation(
                            p_r[:, ts(jq, jt)], s_ps[:], AF.Exp,
                            scale=mask_sc[:, ic:ic + 1],
                        )

                def emit_transpose_group(ic, g):
                    p_r = p_tiles[ic]
                    ptp = psPT.tile([128, 512], dt.float32r, tag="PT")
                    for t in range(4):
                        nc.tensor.transpose(
                            ptp[:, ts(t, 128)],
                            p_r[:, ts(g * 4 + t, 128)],
                            ident_r[:],
                        )
                    cb_out = pt_sb[:, g * 4:g * 4 + 4, ds((ic % 2) * 128, 128)]
                    if g % act_copy_mod == 0:
                        nc.scalar.copy(cb_out, ptp[:])
                    else:
                        nc.vector.tensor_copy(cb_out, ptp[:])

                def emit_transposes(ic):
                    for g in range(nch // 4):
                        emit_transpose_group(ic, g)

                pv_state = {"jc": 0, "ot": None}

                def emit_pv_upto(jc_end):
                    # emit PV accumulation matmuls for jc in [state, jc_end)
                    if pv_state["jc"] == 0 and jc_end > 0:
                        ot_new = psO.tile([DA, 256], dt.float32, tag="obank")
                        pv_state["ot"] = ot_new
                    ot_ps = pv_state["ot"]
                    for jc in range(pv_state["jc"], jc_end):
                        nc.tensor.matmul(
                            ot_ps[:],
                            v_r[:, jc, :],
                            pt_sb[:, jc, :],
                            start=(jc == 0), stop=(jc == nch - 1),
                        )
                    pv_state["jc"] = jc_end

                def emit_pv(ic1):
                    # epilogue: consumes accumulated outT for (ic1-1, ic1)
                    emit_pv_upto(nch)
                    pv_state["jc"] = 0
                    ot_ps = pv_state["ot"]
                    ot_sb = outsb.tile([DA, 256], dt.float32, tag="ot_sb")
                    nc.vector.tensor_copy(ot_sb[:], ot_ps[:])
                    for t in range(2):
                        i_abs = (ic1 - 1) + t
                        fix = psO.tile([128, DA], dt.float32, tag="obank")
                        nc.tensor.transpose(
                            fix[:], ot_sb[:, ts(t, 128)],
                            ident_f[0:DA, 0:DA],
                        )
                        recz = stats.tile([128, 1], dt.float32, tag="recz")
                        nc.vector.reciprocal(recz[:], fix[:, D:DA])
                        o_sb = outsb.tile([128, D], dt.float32, tag="o_sb")
                        nc.vector.tensor_scalar_mul(o_sb[:], fix[:, 0:D], recz[:])
                        nc.sync.dma_start(out_d[h, i_abs], o_sb[:])
                        pn = pnorm.tile([128, n_ctx], dt.float32, tag="pn")
                        nc.vector.tensor_scalar_mul(
                            pn[:], p_tiles[i_abs][:], recz[:]
                        )
                        nc.sync.dma_start(pat_d[h, i_abs], pn[:])
                    del p_tiles[ic1 - 1], p_tiles[ic1]

                # Interleave PV chunks (HAM-counted matmuls) between
                # transpose groups (transpose-mode is not HAM-counted) and
                # spread each pair's PV over BOTH following transpose
                # stretches: first half during the odd i-chunk, second half
                # during the next even i-chunk.
                ng = nch // 4
                for ic in range(nch):
                    emit_qk_exp(ic)
                    for g in range(ng):
                        emit_transpose_group(ic, g)
                        if ic % 2 == 1:
                            emit_pv_upto(4 * g)
                    if ic % 2 == 1:
                        emit_pv(ic)

    nc.compile()
    return nc


_cache = {}


def get_nc(n_ctx=N, heads=HPC):
    key = (n_ctx, heads)
    if key not in _cache:
        _cache[key] = build_nc(n_ctx, heads)
    return _cache[key]


def _augment(x, aug_col):
    """[HPC, n, D] + [HPC, n] -> [HPC, 128, nch, D+1] chunk layout."""
    hpc, n, d = x.shape
    nch = n // 128
    xa = np.concatenate([x, aug_col[:, :, None].astype(np.float32)], axis=-1)
    return np.ascontiguousarray(
        xa.reshape(hpc, nch, 128, d + 1).transpose(0, 2, 1, 3)
    )


def make_in_maps(query, key, value, mask):
    q = np.asarray(query, dtype=np.float32).reshape(B * H, N, D)
    k = np.asarray(key, dtype=np.float32).reshape(B * H, N, D)
    v = np.asarray(value, dtype=np.float32).reshape(B * H, N, D)
    m = np.asarray(mask, dtype=np.int32)
    nch = N // 128

    ones = np.ones((HPC, N), np.float32)
    in_maps = []
    for c in range(N_CORES):
        gs = [HPC * c + i for i in range(HPC)]
        sl = slice(gs[0], gs[-1] + 1)
        mh = np.stack([m[g // H] for g in gs])  # [HPC, N] int32
        mbias = (mh.astype(np.float32) - 1.0) * 8.0e9
        in_maps.append({
            "q": _augment(q[sl], ones),
            "k": _augment(k[sl], mbias),
            "v": _augment(v[sl], ones),
            "m": np.ascontiguousarray(mh.reshape(HPC, nch, 128)),
        })
    return in_maps


def gather_results(res):
    out = np.concatenate([r["out"] for r in res]).reshape(B, H, N, D)
    pat = np.concatenate([r["pat"] for r in res]).reshape(B, H, N, N)
    return out, pat


def kernel(query, key, value, mask):
    nc = get_nc()
    in_maps = make_in_maps(query, key, value, mask)
    res = run_bass_kernel_spmd(nc, in_maps, core_ids=list(range(N_CORES))).results
    return gather_results(res)


# revision 16
# speedup vs baseline: 1.1804x; 1.0784x over previous
"""Masked attention (out, p_attn) kernel for Trainium2, 8-core SPMD.

Problem: B=2 H=8 N=4096 D=64 fp32 attention with outer-product int mask,
returning BOTH the attention output [B,H,N,D] and the full probability
matrix p_attn [B,H,N,N] (the 1.07 GB p_attn write dominates -> memory
bound, roofline ~400us across 8 cores).

Sharding: the 16 (b,h) heads are split 2-per-core (head-parallel, no
communication).

Per-core pipeline per head (matmul operands are float32r: fp32 storage
rounded so the PE runs 1 cycle/row; ~1.5e-4 matmul rel err vs bf16's
2.3e-3):
  - Host augments Q with a ones column and K with a mask-bias column
    ((mask_j-1)*8e9), so the single QK matmul (K=65 contraction) yields
    raw scores with -1e9 (pre-scale) already added to masked key columns.
    V gets a ones column, so the PV matmul's row 64 accumulates the
    softmax denominator Z for free.
  - Prologue PE-transposes Q,K into d-major [65, N] f32r operands.
  - Per 128-row i-chunk: S tiles in PSUM; ACT exp with per-partition
    scale = 0.125 * mask_i (masked query rows -> exp(0)=1 everywhere ->
    uniform row, matching jax softmax of an all--1e9 row). exp output is
    unnormalized P (f32r).
  - PE transposes P 128x128 -> PSUM; ACT/DVE copy tiles back to SBUF
    (PT). PV matmul outT[65, 256] = sum_j V'[j,:].T @ PT[j, i-range].
  - Fixup: PE-transpose outT -> [128, 65]; col 64 = Z; DVE reciprocal
    gives 1/Z per query row, scales out rows, and normalizes P for the
    p_attn DMA.
"""

import numpy as np

import concourse.bass as bass
import concourse.mybir as mybir
import concourse.tile as tile
from concourse import bacc
from concourse.bass import ds, ts
from concourse.bass_utils import run_bass_kernel_spmd
from concourse.masks import make_identity

dt = mybir.dt
AF = mybir.ActivationFunctionType

B, H, N, D = 2, 8, 4096, 64
N_CORES = 8
HPC = (B * H) // N_CORES  # heads per core
DA = D + 1  # augmented feature dim


def build_nc(n_ctx=N, heads=HPC, act_copy_mod=3):
    """Build + compile the per-core kernel for `heads` heads of [n_ctx, D]."""
    nch = n_ctx // 128
    n_jq = max(1, n_ctx // 1024)
    jt = n_ctx // n_jq
    assert jt <= 1024 and jt % 512 == 0

    nc = bacc.Bacc("TRN2", target_bir_lowering=False, debug=False)
    q_d = nc.dram_tensor("q", [heads, 128, nch, DA], dt.float32, kind="ExternalInput").ap()
    k_d = nc.dram_tensor("k", [heads, 128, nch, DA], dt.float32, kind="ExternalInput").ap()
    v_d = nc.dram_tensor("v", [heads, 128, nch, DA], dt.float32, kind="ExternalInput").ap()
    m_d = nc.dram_tensor("m", [heads, nch, 128], dt.int32, kind="ExternalInput").ap()
    out_d = nc.dram_tensor("out", [heads, nch, 128, D], dt.float32, kind="ExternalOutput").ap()
    pat_d = nc.dram_tensor("pat", [heads, nch, 128, n_ctx], dt.float32, kind="ExternalOutput").ap()

    with tile.TileContext(nc) as tc:
        with (
            tc.tile_pool(name="const", bufs=1) as constp,
            tc.tile_pool(name="loads", bufs=1) as loads,
            tc.tile_pool(name="vr", bufs=1) as vrp,
            tc.tile_pool(name="prob", bufs=3) as prob,
            tc.tile_pool(name="pnorm", bufs=2) as pnorm,
            tc.tile_pool(name="ptsb", bufs=1) as ptsb,
            tc.tile_pool(name="stats", bufs=2) as stats,
            tc.tile_pool(name="outsb", bufs=2) as outsb,
            tc.tile_pool(name="psS", bufs=2, space="PSUM") as psS,
            tc.tile_pool(name="psPT", bufs=3, space="PSUM") as psPT,
            tc.tile_pool(name="psO", bufs=1, space="PSUM") as psO,
        ):
            ident_f = constp.tile([128, 128], dt.float32)
            make_identity(nc, ident_f[:])
            ident_r = constp.tile([128, 128], dt.float32r)
            nc.vector.tensor_copy(ident_r[:], ident_f[:])

            # K=128-padded QK operands: rows 0-64 written per head, rows
            # 64-127 zeroed once (zero rows make the f32r matmul take the
            # fast full-K path: ~1 cyc/row vs 2 at K=65)
            qt = constp.tile([128, n_ctx], dt.float32r)
            kt = constp.tile([128, n_ctx], dt.float32r)
            nc.vector.memset(qt[64:128, :].bitcast(dt.float32), 0.0)
            nc.vector.memset(kt[64:128, :].bitcast(dt.float32), 0.0)

            for h in range(heads):
                # ---- loads ----
                qf = loads.tile([128, nch, DA], dt.float32, tag="qf")
                kf = loads.tile([128, nch, DA], dt.float32, tag="kf")
                vf = loads.tile([128, nch, DA], dt.float32, tag="vf")
                nc.sync.dma_start(qf[:], q_d[h])
                nc.sync.dma_start(kf[:], k_d[h])
                nc.sync.dma_start(vf[:], v_d[h])
                m32 = loads.tile([nch, 128], dt.int32, tag="m32")
                nc.sync.dma_start(m32[:], m_d[h])

                # ---- operand prep ----
                v_r = vrp.tile([128, nch, DA], dt.float32r, tag="vr")
                nc.vector.tensor_copy(v_r[:], vf[:])

                for src, dst in ((qf, qt), (kf, kt)):
                    for g in range(nch // 4):
                        ptile = psPT.tile([DA, 512], dt.float32, tag="PT")
                        for t in range(4):
                            c = g * 4 + t
                            nc.tensor.transpose(
                                ptile[:, ts(t, 128)], src[:, c, :], ident_f[:]
                            )
                        nc.vector.tensor_copy(dst[0:DA, ts(g, 512)], ptile[:])

                # mask_sc[p, ic] = 0.125 * mask_i  (per-partition exp scale)
                m32f = stats.tile([nch, 128], dt.float32, tag="m32f")
                nc.vector.tensor_copy(m32f[:], m32[:])
                mfix = psO.tile([128, DA], dt.float32, tag="obank")
                nc.tensor.transpose(
                    mfix[:, 0:nch], m32f[:], ident_f[0:nch, 0:nch]
                )
                mask_sc = stats.tile([128, nch], dt.float32, tag="mask_sc")
                nc.vector.tensor_scalar_mul(mask_sc[:], mfix[:, 0:nch], 0.125)

                pt_sb = ptsb.tile([128, nch, 256], dt.float32r, tag="pt")
                p_tiles = {}

                def emit_qk_exp(ic):
                    p_r = prob.tile([128, n_ctx], dt.float32r, tag="p")
                    p_tiles[ic] = p_r
                    for jq in range(n_jq):
                        s_ps = psS.tile([128, jt], dt.float32, tag="S")
                        for js in range(jt // 512):
                            nc.tensor.matmul(
                                s_ps[:, ts(js, 512)],
                                qt[:, ts(ic, 128)],
                                kt[:, ds(jq * jt + js * 512, 512)],
                                start=True, stop=True,
                            )
                        nc.scalar.activation(
                            p_r[:, ts(jq, jt)], s_ps[:], AF.Exp,
                            scale=mask_sc[:, ic:ic + 1],
                        )

                def emit_transpose_group(ic, g):
                    p_r = p_tiles[ic]
                    ptp = psPT.tile([128, 512], dt.float32r, tag="PT")
                    for t in range(4):
                        nc.tensor.transpose(
                            ptp[:, ts(t, 128)],
                            p_r[:, ts(g * 4 + t, 128)],
                            ident_r[:],
                        )
                    cb_out = pt_sb[:, g * 4:g * 4 + 4, ds((ic % 2) * 128, 128)]
                    if g % act_copy_mod == 0:
                        nc.scalar.copy(cb_out, ptp[:])
                    else:
                        nc.vector.tensor_copy(cb_out, ptp[:])

                def emit_transposes(ic):
                    for g in range(nch // 4):
                        emit_transpose_group(ic, g)

                pv_state = {"jc": 0, "ot": None}

                def emit_pv_upto(jc_end):
                    # emit PV accumulation matmuls for jc in [state, jc_end)
                    if pv_state["jc"] == 0 and jc_end > 0:
                        ot_new = psO.tile([DA, 256], dt.float32, tag="obank")
                        pv_state["ot"] = ot_new
                    ot_ps = pv_state["ot"]
                    for jc in range(pv_state["jc"], jc_end):
                        nc.tensor.matmul(
                            ot_ps[:],
                            v_r[:, jc, :],
                            pt_sb[:, jc, :],
                            start=(jc == 0), stop=(jc == nch - 1),
                        )
                    pv_state["jc"] = jc_end

                def emit_pv(ic1):
                    # epilogue: consumes accumulated outT for (ic1-1, ic1)
                    emit_pv_upto(nch)
                    pv_state["jc"] = 0
                    ot_ps = pv_state["ot"]
                    ot_sb = outsb.tile([DA, 256], dt.float32, tag="ot_sb")
                    nc.vector.tensor_copy(ot_sb[:], ot_ps[:])
                    for t in range(2):
                        i_abs = (ic1 - 1) + t
                        fix = psO.tile([128, DA], dt.float32, tag="obank")
                        nc.tensor.transpose(
                            fix[:], ot_sb[:, ts(t, 128)],
                            ident_f[0:DA, 0:DA],
                        )
                        recz = stats.tile([128, 1], dt.float32, tag="recz")
                        nc.vector.reciprocal(recz[:], fix[:, D:DA])
                        o_sb = outsb.tile([128, D], dt.float32, tag="o_sb")
                        nc.vector.tensor_scalar_mul(o_sb[:], fix[:, 0:D], recz[:])
                        nc.sync.dma_start(out_d[h, i_abs], o_sb[:])
                        pn = pnorm.tile([128, n_ctx], dt.float32, tag="pn")
                        nc.vector.tensor_scalar_mul(
                            pn[:], p_tiles[i_abs][:], recz[:]
                        )
                        nc.sync.dma_start(pat_d[h, i_abs], pn[:])
                    del p_tiles[ic1 - 1], p_tiles[ic1]

                # Interleave PV chunks (HAM-counted matmuls) between
                # transpose groups (transpose-mode is not HAM-counted) and
                # spread each pair's PV over BOTH following transpose
                # stretches: first half during the odd i-chunk, second half
                # during the next even i-chunk.
                ng = nch // 4
                for ic in range(nch):
                    emit_qk_exp(ic)
                    for g in range(ng):
                        emit_transpose_group(ic, g)
                        if ic % 2 == 1:
                            emit_pv_upto(4 * g)
                    if ic % 2 == 1:
                        emit_pv(ic)

    nc.compile()
    return nc


_cache = {}


def get_nc(n_ctx=N, heads=HPC):
    key = (n_ctx, heads)
    if key not in _cache:
        _cache[key] = build_nc(n_ctx, heads)
    return _cache[key]


def _augment(x, aug_col):
    """[HPC, n, D] + [HPC, n] -> [HPC, 128, nch, D+1] chunk layout."""
    hpc, n, d = x.shape
    nch = n // 128
    xa = np.concatenate([x, aug_col[:, :, None].astype(np.float32)], axis=-1)
    return np.ascontiguousarray(
        xa.reshape(hpc, nch, 128, d + 1).transpose(0, 2, 1, 3)
    )


def make_in_maps(query, key, value, mask):
    q = np.asarray(query, dtype=np.float32).reshape(B * H, N, D)
    k = np.asarray(key, dtype=np.float32).reshape(B * H, N, D)
    v = np.asarray(value, dtype=np.float32).reshape(B * H, N, D)
    m = np.asarray(mask, dtype=np.int32)
    nch = N // 128

    ones = np.ones((HPC, N), np.float32)
    in_maps = []
    for c in range(N_CORES):
        gs = [HPC * c + i for i in range(HPC)]
        sl = slice(gs[0], gs[-1] + 1)
        mh = np.stack([m[g // H] for g in gs])  # [HPC, N] int32
        mbias = (mh.astype(np.float32) - 1.0) * 8.0e9
        in_maps.append({
            "q": _augment(q[sl], ones),
            "k": _augment(k[sl], mbias),
            "v": _augment(v[sl], ones),
            "m": np.ascontiguousarray(mh.reshape(HPC, nch, 128)),
        })
    return in_maps


def gather_results(res):
    out = np.concatenate([r["out"] for r in res]).reshape(B, H, N, D)
    pat = np.concatenate([r["pat"] for r in res]).reshape(B, H, N, N)
    return out, pat


def kernel(query, key, value, mask):
    nc = get_nc()
    in_maps = make_in_maps(query, key, value, mask)
    res = run_bass_kernel_spmd(nc, in_maps, core_ids=list(range(N_CORES))).results
    return gather_results(res)
